# revision 1
# baseline (speedup 1.0000x reference)
"""Binary-split tree decoder on Trainium2 (Bass/Tile), 8-core data-parallel.

alphas [1_000_000, 127] f32 -> out [1_000_000, 256] f32.

out[:, 0] = 1; for heap node j in [1, 255): out[:, j] = out[:, (j-1)//2] *
(alphas[:, (j-1)//2] if j odd else 1 - alphas[:, (j-1)//2]); out[:, 255] = 0.

Sharding: batch dim split evenly across the 8 NeuronCores (no cross-device
communication). Per core, rows are processed in blocks of P=128 partitions x
R rows-per-partition: partition p holds R *consecutive* DRAM rows side by
side in the free dim, so every DMA is a single contiguous chunk per
partition. The tree levels are computed in place in the output tile: per
level one tensor_mul writes the left children (stride-2 AP) and one
tensor_sub (parent - left = parent * (1 - a)) writes the right children.
"""

import sys

for _p in ("/root/.axon_site/_ro/trn_rl_repo", "/opt/trn_rl_repo"):
    if _p not in sys.path:
        sys.path.append(_p)

import contextlib

import numpy as np

import concourse.bass as bass
import concourse.tile as tile
from concourse import mybir
from concourse.bass_utils import run_bass_kernel_spmd

B = 1_000_000
C_IN = 127
C_OUT = 256
DEPTH = 8
N_CORES = 8
ROWS_PER_CORE = B // N_CORES  # 125_000
R_GROUPS = 32  # rows per partition per block (128*32 = 4096 rows/block)
F32 = mybir.dt.float32


def _split_waits(nc):
    """This walrus build rejects >1 sync-wait condition per instruction
    ("Too many sync wait commands"). Hoist extra waits onto single-wait
    NoOps inserted just before the instruction on the same engine."""
    uid = 0
    for fn in nc.m.functions:
        for bb in fn.blocks:
            new = []
            changed = False
            for ins in bb.instructions:
                si = ins.sync_info
                if si is not None and si.on_wait is not None and len(si.on_wait) > 1:
                    waits = list(si.on_wait)
                    for w in waits[:-1]:
                        nop = mybir.InstNoOp(name=f"wait_split_{uid}", ins=[], outs=[])
                        uid += 1
                        nop.engine = ins.engine
                        nop.sync_info = mybir.SyncInfo(on_wait=[w], on_update=[])
                        new.append(nop)
                    si.on_wait = waits[-1:]
                    ins.sync_info = si
                    changed = True
                new.append(ins)
            if changed:
                bb.instructions = new


@contextlib.contextmanager
def _maybe_trim_exit(trim: bool):
    """Optionally drop the second all-engine barrier of the Tile exit
    sequence: it orders the semaphore clears against nothing (engines halt
    independently after their last instruction; no cross-core sync)."""
    if not trim:
        yield
        return
    from concourse.vector_clock import ScopedClock

    orig = tile.TileContext._drain_and_barrier

    def patched(self, tick_clock, wait_clock):
        nc = self.nc
        drain_inst = nc.sync.drain()
        wait_clock.add_sem_waits(
            drain_inst.ins, ScopedClock({None: tick_clock.global_clock})
        )
        nc.all_engine_barrier()
        popped = nc._tile_sem_poison_stack.pop()
        assert popped is self._sem_poison
        nc.clear_and_free_semaphores(list(self.sems.allocated().values()))

    tile.TileContext._drain_and_barrier = patched
    try:
        yield
    finally:
        tile.TileContext._drain_and_barrier = orig


def _blocks(rows: int, r_groups: int, ramp: tuple = ()):
    """Split `rows` into (start, P, R) blocks: optional small ramp-up blocks
    (so compute/stores start early), then full 128 x r_groups blocks, then a
    128 x (rem//128) block, then a partial-partition tail."""
    out = []
    s = 0
    for r in ramp:
        if rows - s >= 128 * r:
            out.append((s, 128, r))
            s += 128 * r
    while s < rows:
        rem = rows - s
        if rem >= 128 * r_groups:
            p, r = 128, r_groups
        elif rem >= 128:
            p, r = 128, rem // 128
        else:
            p, r = rem, 1
        out.append((s, p, r))
        s += p * r
    return out


def build_nc(
    rows: int = ROWS_PER_CORE,
    r_groups: int = R_GROUPS,
    bufs: int = 3,
    ramp: tuple = (),
    in_bufs: int | None = None,
    out_bufs: int | None = None,
    swap_rings: bool = False,
    third_ring: bool = False,
    trim_exit: bool = False,
):
    """Build the per-core Bass program: alphas [rows,127] -> out [rows,256]."""
    nc = bass.Bass("TRN2", target_bir_lowering=False, debug=False)
    a = nc.declare_dram_parameter("alphas", [rows, C_IN], F32, isOutput=False)
    o = nc.declare_dram_parameter("out", [rows, C_OUT], F32, isOutput=True)
    load_eng = nc.scalar if swap_rings else nc.sync
    store_eng = nc.sync if swap_rings else nc.scalar

    with _maybe_trim_exit(trim_exit), tile.TileContext(nc) as tc:
        with (
            tc.tile_pool(name="pin", bufs=in_bufs or bufs) as pin,
            tc.tile_pool(name="pout", bufs=out_bufs or bufs) as pout,
        ):
            for bi, (s, p, r) in enumerate(_blocks(rows, r_groups, ramp)):
                if third_ring:
                    store_eng = nc.scalar if bi % 2 == 0 else nc.gpsimd
                tin = pin.tile([p, r * C_IN], F32, tag="tin")
                av = tin[:, :].rearrange("p (r c) -> p r c", c=C_IN)
                load_eng.dma_start(
                    out=av,
                    in_=a[s : s + p * r].rearrange("(p r) c -> p r c", r=r),
                )

                tout = pout.tile([p, r * C_OUT], F32, tag="tout")
                ov = tout[:, :].rearrange("p (r c) -> p r c", c=C_OUT)
                nc.vector.memset(ov[:, :, 0:1], 1.0)
                nc.vector.memset(ov[:, :, C_OUT - 1 : C_OUT], 0.0)
                for d in range(DEPTH - 1):
                    n = 1 << d
                    parent = ov[:, :, n - 1 : 2 * n - 1]
                    alpha = av[:, :, n - 1 : 2 * n - 1]
                    left = ov[:, :, 2 * n - 1 : 4 * n - 2 : 2]
                    right = ov[:, :, 2 * n : 4 * n - 1 : 2]
                    nc.vector.tensor_mul(left, parent, alpha)
                    nc.vector.tensor_sub(right, parent, left)

                store_eng.dma_start(
                    out=o[s : s + p * r].rearrange("(p r) c -> p r c", r=r),
                    in_=ov,
                )
    _split_waits(nc)
    return nc


_NC_CACHE: dict = {}


def _get_nc(rows: int):
    if rows not in _NC_CACHE:
        _NC_CACHE[rows] = build_nc(rows)
    return _NC_CACHE[rows]


def make_in_maps(alphas: np.ndarray):
    rows = alphas.shape[0] // N_CORES
    return [
        {"alphas": np.ascontiguousarray(alphas[i * rows : (i + 1) * rows])}
        for i in range(N_CORES)
    ]


def kernel(alphas: np.ndarray) -> np.ndarray:
    alphas = np.asarray(alphas, dtype=np.float32)
    assert alphas.shape == (B, C_IN), alphas.shape
    nc = _get_nc(ROWS_PER_CORE)
    res = run_bass_kernel_spmd(
        nc, make_in_maps(alphas), core_ids=list(range(N_CORES))
    )
    return np.concatenate([res.results[i]["out"] for i in range(N_CORES)], axis=0)

